# revision 1
# baseline (speedup 1.0000x reference)
"""Trainium2 Bass kernel for BilinearDecoder.

score = sigmoid( einsum('ed,ed->e', z[edges[0]] @ W, z[edges[1]]) )

Strategy (row-band edge sharding across 8 cores):
  Host globally row-sorts the 131072 edges; core c owns the c-th
  contiguous slice of 16384, so its rows span ~N_NODES/8 nodes and it
  receives a rebased 2048-node zT shard as input.

  Phase 1 (done by ~32us): zw_shard = z_shard @ W (2048x512 @ 512x512,
  f16 in / f32 psum), cast by ACT straight into an SBUF table — zw
  never touches DRAM.

  Phase 2: per 1024-edge chunk, a SBUF-source non-transpose dma_gather
  pulls zw_shard[row] rows (built via raw InstDMAGatherAnt; the Q7
  ucode supports this mode though bass asserts transpose-only) and a
  DRAM dma_gather pulls z[col] rows from the global table.  Gathers
  rotate across all 4 SWDGE queues, nosync-chained so the scheduled
  order keeps Tile's DMASW sem-lane/queue binding consistent; row
  chunks gate on rank-prefix slices of the zw table so they overlap
  phase 1.  Per 128-edge block: DVE f16 mul, then the free-dim dot
  reduce alternates DVE tensor_reduce / ACT accumulator.  One sigmoid,
  one DMA out; host unsorts globally.
"""

import sys

if "/opt/trn_rl_repo" not in sys.path:
    sys.path.insert(0, "/opt/trn_rl_repo")

import numpy as np

N_NODES = 10000
N_NODES_PAD = 10240  # pad to multiple of 128
W_DIM = 512
N_EDGES = 131072
N_CORES = 8
EC = N_EDGES // N_CORES  # 16384 edges per core
CHUNK = 1024  # edges per dma_gather (2048 overflows the SWDGE desc ring)
NCHUNK = EC // CHUNK  # 16
NBLK = EC // 128  # 128 score columns per core
NQ = 4  # SWDGE queues
SHARD = 2048  # zw node-range per core (edges are row-band sharded; a core's
              # 16384 globally-row-sorted edges span ~1250 nodes << 2048)

_cache = {}


def _chunk_bounds():
    """Static per-chunk zw prefix bounds (in nodes, multiple of 128).

    Host sorts each core's edges by row index, so row-gather chunk k only
    reads zw rows below roughly the (k+1)/NCHUNK quantile.  The +768
    margin is >10 sigma of the order-statistic fluctuation for uniform
    indices; the host verifies per input and falls back to full bounds."""
    bs = []
    for k in range(NCHUNK):
        b = int(np.ceil((N_NODES_PAD * (k + 1) / NCHUNK + 768) / 128.0) * 128)
        bs.append(min(N_NODES_PAD, b))
    return bs


def _build(bounds):
    import concourse.bacc as bacc
    import concourse.tile as tile
    from concourse import mybir

    from concourse.bass import InstructionNameOrderedSet

    f32 = mybir.dt.float32
    f16 = mybir.dt.float16
    i16 = mybir.dt.int16

    nc = bacc.Bacc(
        "TRN2",
        target_bir_lowering=False,
        debug=False,
        num_devices=N_CORES,
        num_swdge_queues=NQ,
    )
    # zt4[p, k, n] = z_shard[n, k*128+p]: one dma_start loads a whole lhsT
    # panel.  Each core receives only its 2048-node row-band shard.
    zt = nc.dram_tensor("zt", [128, 4, SHARD], f16, kind="ExternalInput")
    ztbl = nc.dram_tensor("ztbl", [N_NODES_PAD, W_DIM], f16, kind="ExternalInput")
    w = nc.dram_tensor("w", [W_DIM, W_DIM], f16, kind="ExternalInput")
    ridx = nc.dram_tensor("ridx", [128, EC // 16], i16, kind="ExternalInput")
    cidx = nc.dram_tensor("cidx", [128, EC // 16], i16, kind="ExternalInput")
    out = nc.dram_tensor("scores", [128, NBLK], f32, kind="ExternalOutput")

    with tile.TileContext(nc) as tc:
        with (
            tc.tile_pool(name="wpool", bufs=1) as wpool,
            tc.tile_pool(name="zpanel", bufs=3) as zpool,
            tc.tile_pool(name="zwtbl", bufs=1) as zwpool,
            tc.tile_pool(name="psum", bufs=6, space="PSUM") as psum_pool,
            tc.tile_pool(name="idx", bufs=1) as idxpool,
            tc.tile_pool(name="rgath", bufs=8) as rpool,
            tc.tile_pool(name="cgath", bufs=8) as cpool,
            tc.tile_pool(name="scr", bufs=8) as spool,
            tc.tile_pool(name="misc", bufs=1) as mpool,
        ):
            PANEL = 1024  # nodes per zT panel, loaded by ONE dma_start

            # Issue order = latency (~0.65us SP issue overhead per DMA):
            # cidx gates the lead col gathers, then the first zT panel + W
            # gate the first matmul; ridx is needed only by the first row
            # gather (~29us).
            cidx_sb = idxpool.tile([128, EC // 16], i16, tag="cidx")
            nc.sync.dma_start(cidx_sb[:], cidx[:])
            zp_first = zpool.tile([128, 4, PANEL], f16, tag="zp")
            nc.sync.dma_start(zp_first[:], zt[:, :, :PANEL])
            w_tiles = []
            for k in range(4):
                wt = wpool.tile([128, W_DIM], f16, tag=f"w{k}")
                nc.sync.dma_start(wt[:], w[k * 128 : (k + 1) * 128, :])
                w_tiles.append(wt)
            ridx_sb = idxpool.tile([128, EC // 16], i16, tag="ridx")
            nc.sync.dma_start(ridx_sb[:], ridx[:])

            scores = mpool.tile([128, NBLK], f32, tag="scores")

            # Dummy sigmoid first: makes the ACT-table pass pick a function
            # set containing BOTH sigmoid and copy, so the final sigmoid
            # needs no table reload on the critical tail.
            warm = mpool.tile([128, 1], f32, tag="warm")
            nc.scalar.activation(
                warm[:], w_tiles[0][:, 0:1], mybir.ActivationFunctionType.Sigmoid
            )

            # SBUF-resident zW table: node n at partition n%128, rank n//128.
            # Phase 1 writes it straight from PSUM (no DRAM round-trip);
            # row gathers read it via SBUF-source dma_gather.
            zw_sb = zwpool.tile([128, SHARD // 128, W_DIM], f16, tag="zw")

            # ---- Phase 1: zw shard = z_shard @ W ----
            for p in range(SHARD // PANEL):
                if p == 0:
                    zp = zp_first
                else:
                    zp = zpool.tile([128, 4, PANEL], f16, tag="zp")
                    nc.sync.dma_start(
                        zp[:], zt[:, :, p * PANEL : (p + 1) * PANEL]
                    )
                for ntile in range(PANEL // 128):
                    ps = psum_pool.tile([128, W_DIM], f32, tag="ps")
                    for k in range(4):
                        nc.tensor.matmul(
                            ps[:],
                            lhsT=zp[:, k, ntile * 128 : (ntile + 1) * 128],
                            rhs=w_tiles[k][:],
                            start=(k == 0),
                            stop=(k == 3),
                        )
                    rank = p * (PANEL // 128) + ntile
                    nc.scalar.activation(
                        zw_sb[:, rank, :],
                        ps[:],
                        mybir.ActivationFunctionType.Copy,
                    )

            # ---- Phase 2: gather + per-edge dot ----
            # All SWDGE gathers rotate queues 0..3.  Tile's 8 DMASW sem
            # lanes advance round-robin over Pool DMAs in SCHEDULED order
            # and each lane hard-locks to one queue, so the gathers are
            # nosync-chained to pin their scheduled order to emission
            # order (the gpsimd engine is in-order regardless); queue =
            # position % 4 then keeps lane L on queue L%4 forever.  Col
            # gathers lead row gathers so the 4 Q7 queue pairs have work
            # while phase 1 is still filling zw.
            gstate = {"pos": 0, "prev": None}

            def _chain(inst):
                if gstate["prev"] is not None:
                    deps = InstructionNameOrderedSet()
                    deps.add(gstate["prev"])
                    inst.ins.add_nosync_dependencies_from(deps)
                gstate["prev"] = inst.ins.name
                gstate["pos"] += 1
                return inst

            def emit_gather(dst, src, idxs):
                _chain(
                    nc.gpsimd.dma_gather(
                        dst, src, idxs, CHUNK, CHUNK, W_DIM,
                        queue_num=gstate["pos"] % NQ,
                    )
                )

            def emit_sbuf_gather(dst, src, idxs):
                # SBUF-source NON-transpose gather: the Q7 ucode's gen_descs
                # handles src_is_sbuf independently of transpose (validated
                # on HW), but bass.dma_gather asserts transpose for SBUF
                # sources, so build the instruction directly.
                g = nc.gpsimd
                inst = g.add_instruction(
                    mybir.InstDMAGatherAnt(
                        name=nc.get_next_instruction_name(),
                        ins=[
                            g.lower_ap(src),
                            g.lower_ap(idxs),
                            g.lower_val_access(g.to_reg(CHUNK)),
                        ],
                        outs=[g.lower_ap(dst)],
                        transpose=False,
                        num_idxs=CHUNK,
                        elem_size=W_DIM,
                        stride_bytes_256=0,
                        gen_mode=0,
                        single_packet=True,
                        queue_num=gstate["pos"] % NQ,
                        sbuf_tokens_per_rank=128,
                        sbuf_free_dim_per_rank=W_DIM * 2,
                        sbuf_free_dim_pad_per_rank=0,
                        sbuf_byte_offset=0,
                    )
                )
                _chain(inst)

            row_tiles = {}
            col_tiles = {}

            def emit_row(ch):
                rt = rpool.tile([128, CHUNK // 128, W_DIM], f16, tag="rowbuf")
                icol = slice(ch * (CHUNK // 16), (ch + 1) * (CHUNK // 16))
                # Rank-prefix-sliced zw view: the gather only depends on the
                # phase-1 casts that write ranks < bounds[ch]/128, so row
                # gathers overlap the phase-1 matmul.
                emit_sbuf_gather(
                    rt[:], zw_sb[:, : bounds[ch] // 128, :], ridx_sb[:, icol]
                )
                row_tiles[ch] = rt

            def emit_col(ch):
                ct = cpool.tile([128, CHUNK // 128, W_DIM], f16, tag="colbuf")
                icol = slice(ch * (CHUNK // 16), (ch + 1) * (CHUNK // 16))
                emit_gather(ct[:], ztbl[:], cidx_sb[:, icol])
                col_tiles[ch] = ct

            def emit_dot(ch):
                rt, ct = row_tiles.pop(ch), col_tiles.pop(ch)
                # Dot work stays on DVE while phase 1 runs: ACT's in-order
                # queue must hold only phase-1 casts then, or gather-gated
                # dots convoy the casts (and with them phase 1 + the row
                # gathers) to gather pace.  The last chunks run after the
                # casts are done, so their reduces split DVE/ACT to shrink
                # the tail backlog.
                for b in range(CHUNK // 128):
                    col = ch * (CHUNK // 128) + b
                    scr = spool.tile([128, W_DIM], f16, tag="ttr")
                    nc.vector.tensor_mul(scr[:], rt[:, b, :], ct[:, b, :])
                    if b % 2 == 1:
                        dump = spool.tile([128, W_DIM], f16, tag="dump")
                        nc.scalar.activation(
                            dump[:],
                            scr[:],
                            mybir.ActivationFunctionType.Copy,
                            accum_out=scores[:, col : col + 1],
                        )
                    else:
                        nc.vector.tensor_reduce(
                            scores[:, col : col + 1],
                            scr[:],
                            mybir.AxisListType.X,
                            mybir.AluOpType.add,
                        )

            # Two cols lead (no deps, start the stream immediately); then
            # rows interleave, each row's zw-bound wait throttling the
            # in-order gather stream to phase-1 pace.
            LEAD = 2
            for ch in range(LEAD):
                emit_col(ch)
            for ch in range(NCHUNK):
                emit_row(ch)
                if ch + LEAD < NCHUNK:
                    emit_col(ch + LEAD)
                emit_dot(ch)

            sig = mpool.tile([128, NBLK], f32, tag="sig")
            nc.scalar.activation(
                sig[:], scores[:], mybir.ActivationFunctionType.Sigmoid
            )
            nc.sync.dma_start(out[:], sig[:])

    nc.compile()
    return nc


def _get_nc(bounds):
    key = "nc_" + ",".join(map(str, bounds))
    if key not in _cache:
        _cache[key] = _build(bounds)
    return _cache[key]


def _wrap_idx(idx):
    """int16 indices -> [128, n/16] layout: index i at [i%16, i//16],
    replicated across the 8 GPSIMD core groups (16 partitions each)."""
    blk = idx.reshape(-1, 16).T.astype(np.int16)  # [16, n/16]
    return np.ascontiguousarray(np.tile(blk, (8, 1)))  # [128, n/16]


def kernel(z, batch_edges, W, _profile=False):
    from concourse.bass_utils import run_bass_kernel_spmd

    z = np.asarray(z, dtype=np.float32)
    W = np.asarray(W, dtype=np.float32)
    be = np.asarray(batch_edges)

    z_pad = np.zeros((N_NODES_PAD, W_DIM), dtype=np.float32)
    z_pad[:N_NODES] = z
    z16 = z_pad.astype(np.float16)
    w_np = W.astype(np.float16)

    rows_all = be[0].astype(np.int32)
    cols_all = be[1].astype(np.int32)
    # Row-band sharding: core c owns the c-th contiguous slice of the
    # globally row-sorted edges, so its rows span ~N_NODES/8 << SHARD nodes
    # and phase 1 only computes that zw shard.
    glob_order = np.argsort(rows_all, kind="stable")

    bounds = [0] * NCHUNK
    in_maps = []
    for c in range(N_CORES):
        sel = glob_order[c * EC : (c + 1) * EC]
        r_s = rows_all[sel]  # ascending (slice of a global sort)
        c_s = cols_all[sel]
        base = (int(r_s[0]) // 128) * 128
        span = int(r_s[-1]) + 1 - base
        assert span <= SHARD, f"row span {span} exceeds SHARD={SHARD}"
        r_loc = (r_s - base).astype(np.int16)
        for k in range(NCHUNK):
            m = int(r_loc[k * CHUNK : (k + 1) * CHUNK].max()) + 1 + 256
            bounds[k] = max(bounds[k], min(SHARD, (m + 127) // 128 * 128))
        zs = np.zeros((SHARD, W_DIM), np.float16)
        hi = min(base + SHARD, N_NODES_PAD)
        zs[: hi - base] = z16[base:hi]
        zt_np = np.ascontiguousarray(zs.reshape(SHARD, 4, 128).transpose(2, 1, 0))
        in_maps.append(
            {
                "zt": zt_np,
                "ztbl": z16,
                "w": w_np,
                "ridx": _wrap_idx(r_loc),
                "cidx": _wrap_idx(c_s.astype(np.int16)),
            }
        )
    for k in range(1, NCHUNK):
        bounds[k] = max(bounds[k], bounds[k - 1])

    nc = _get_nc(tuple(bounds))
    kwargs = {}
    if _profile:
        kwargs = {"trace": True}
    res = run_bass_kernel_spmd(nc, in_maps, core_ids=list(range(N_CORES)), **kwargs)
    _cache["last_res"] = res

    # Core c's scores are its globally-sorted slice: edge i at [i%128, i//128].
    chunks = []
    for c in range(N_CORES):
        sc = res.results[c]["scores"]
        chunks.append(np.ascontiguousarray(sc.T).reshape(-1))
    full = np.empty(N_EDGES, dtype=np.float32)
    full[glob_order] = np.concatenate(chunks)
    return full

